# revision 1
# baseline (speedup 1.0000x reference)
"""Trainium2 Bass kernel for InpaintingAttnProcessor (3-branch masked SDPA).

Block-sparse formulation: the attention masks depend only on 4 entity
labels, so after sorting tokens by (label, inpainting_bit) on the host,
all three SDPA branches become block-diagonal (the "outside" branch
additionally restricts keys to the im==0 prefix of each block).  Each
core computes one head of the two 8-head branches over all blocks, plus
an entity-aligned slice of the single-head d=640 branch.  Per-entity
bf16 ReduceScatters of the Wo partial products overlap the remaining
compute; the entity branch and the residual are assembled on the host.
"""
import numpy as np
import ml_dtypes
from contextlib import ExitStack

import concourse.bass as bass
import concourse.tile as tile
from concourse import bacc, mybir
from concourse.bass_utils import run_bass_kernel_spmd

S, C, H, D = 4096, 640, 8, 80
NCORES = 8
SCALE_H = 1.0 / np.sqrt(80.0)
SCALE_E = 1.0 / np.sqrt(640.0)
F32 = mybir.dt.float32
BF16 = mybir.dt.bfloat16
BF = ml_dtypes.bfloat16
EXP = mybir.ActivationFunctionType.Exp
COPY = mybir.ActivationFunctionType.Copy
ADD = mybir.AluOpType.add

_cache = {}


def _chunks(total, step=512):
    return [(f0, min(step, total - f0)) for f0 in range(0, total, step)]


def _assign_cores(T):
    """Split entity tiles into NCORES contiguous runs, each within one
    entity. Returns list of (entity, tile0_within_entity, ntiles)."""
    ents = [e for e in range(len(T)) if T[e] > 0]
    c = {e: 1 for e in ents}
    while sum(c.values()) < NCORES:
        e = max(ents, key=lambda x: T[x] / c[x])
        c[e] += 1
    assign = []
    for e in ents:
        base, rem = divmod(T[e], c[e])
        t = 0
        for j in range(c[e]):
            nt = base + (1 if j < rem else 0)
            assign.append((e, t, nt))
            t += nt
    assert len(assign) == NCORES
    return assign


def _build(cfg):
    T, n, n0, assign = cfg
    NE = len(T)
    TEM = max(T)
    NQT = max(a[2] for a in assign)
    Ttot = sum(T)
    Stot = Ttot * 128
    SK, SQ = TEM * 128, NQT * 128
    T0 = [min((x + 127) // 128, T[e]) for e, x in enumerate(n0)]
    off = np.cumsum([0] + [t * 128 for t in T]).tolist()

    nc = bacc.Bacc("TRN2", target_bir_lowering=False, debug=False,
                   num_devices=NCORES)
    d = {}
    d["hT"] = nc.dram_tensor("hT", [C, Stot], BF16, kind="ExternalInput")
    d["whead"] = nc.dram_tensor("whead", [C, 480], BF16, kind="ExternalInput")
    d["went"] = nc.dram_tensor("went", [C, 4 * C], BF16, kind="ExternalInput")
    d["woh"] = nc.dram_tensor("woh", [D, C], BF16, kind="ExternalInput")
    d["hq"] = nc.dram_tensor("hq", [C, SQ], BF16, kind="ExternalInput")
    d["hk"] = nc.dram_tensor("hk", [C, SK], BF16, kind="ExternalInput")
    d["entc"] = nc.dram_tensor("entc", [1, 1], F32, kind="ExternalInput")
    eout_d = nc.dram_tensor("eout", [SQ, C], BF16, kind="ExternalOutput")
    P_d = [nc.dram_tensor(f"P{e}", [128, T[e] * C], BF16) if T[e] else None
           for e in range(NE)]
    red_d = [nc.dram_tensor(f"red{e}", [16, T[e] * C], BF16) if T[e] else None
             for e in range(NE)]
    out_d = [nc.dram_tensor(f"o{e}", [16, T[e] * C], BF16,
                            kind="ExternalOutput") if T[e] else None
             for e in range(NE)]

    with tile.TileContext(nc) as tc:
        _body(nc, tc, d, out_d, red_d, eout_d, P_d, T, T0, n, n0, off,
              TEM, NQT)
    nc.compile()
    return nc


def _body(nc, tc, d, out_d, red_d, eout_d, P_d, T, T0, n, n0, off, TEM,
          NQT):
    NE = len(T)
    Ttot = sum(T)
    Stot = Ttot * 128
    SK, SQ = TEM * 128, NQT * 128
    W4 = 4 * C                      # went row width
    ctx = ExitStack()
    with ctx:
        base = ctx.enter_context(tc.tile_pool(name="base", bufs=1))
        hTb = base.tile([128, 5 * Stot], BF16, tag="hTb")
        wh = base.tile([128, 5 * 480], BF16, tag="wh")
        woh_sb = base.tile([D, C], BF16, tag="woh")
        ones_bf = base.tile([128, 1], BF16, tag="ones_bf")
        ones_f = base.tile([1, 128], F32, tag="ones_f")
        ones_b1 = base.tile([1, 128], BF16, tag="ones_b1")
        entc_sb = base.tile([1, 1], F32, tag="entc")
        nc.vector.memset(ones_bf[:], 1.0)
        nc.vector.memset(ones_f[:], 1.0)
        nc.vector.memset(ones_b1[:], 1.0)
        nc.sync.dma_start(entc_sb[:], d["entc"].ap()[:])
        nc.sync.dma_start(woh_sb[:], d["woh"].ap()[:])
        for cc in range(5):
            nc.sync.dma_start(wh[:, cc * 480:(cc + 1) * 480],
                              d["whead"].ap()[cc * 128:(cc + 1) * 128, :])

        # ================= ENT branch (entity-aligned q slice) ==========
        with tc.tile_pool(name="entp", bufs=1) as ep:
            went = ep.tile([128, 5 * W4], BF16, tag="went")
            hqb = ep.tile([128, 5 * SQ], BF16, tag="hqb")
            hkb = ep.tile([128, 5 * SK], BF16, tag="hkb")
            # load order: q-proj operands first so the PE can start early,
            # then k, v, wof, then the big hT tensor
            for cc in range(5):
                nc.sync.dma_start(
                    went[:, cc * W4:cc * W4 + C],
                    d["went"].ap()[cc * 128:(cc + 1) * 128, 0:C])
                nc.sync.dma_start(hqb[:, cc * SQ:(cc + 1) * SQ],
                                  d["hq"].ap()[cc * 128:(cc + 1) * 128, :])
            for cc in range(5):
                nc.sync.dma_start(
                    went[:, cc * W4 + C:cc * W4 + 2 * C],
                    d["went"].ap()[cc * 128:(cc + 1) * 128, C:2 * C])
                nc.sync.dma_start(hkb[:, cc * SK:(cc + 1) * SK],
                                  d["hk"].ap()[cc * 128:(cc + 1) * 128, :])
            for cc in range(5):
                nc.sync.dma_start(
                    went[:, cc * W4 + 2 * C:(cc + 1) * W4],
                    d["went"].ap()[cc * 128:(cc + 1) * 128, 2 * C:W4])
            for cc in range(5):
                nc.sync.dma_start(hTb[:, cc * Stot:(cc + 1) * Stot],
                                  d["hT"].ap()[cc * 128:(cc + 1) * 128, :])

            qeb = ep.tile([128, 5 * SQ], BF16, tag="qeb")
            keb = ep.tile([128, 5 * SK], BF16, tag="keb")
            veb = ep.tile([128, TEM * C], BF16, tag="veb")
            with tc.tile_pool(name="entqk", bufs=2, space="PSUM") as eps:
                qcs, kcs = _chunks(SQ), _chunks(SK)
                for dc in range(5):
                    pp = eps.tile([128, 1536], F32, tag="ppq", name="ppq")
                    for cc in range(5):
                        for gi, (f0, fw) in enumerate(qcs):
                            nc.tensor.matmul(
                                pp[:, gi * 512:gi * 512 + fw],
                                went[:, cc * W4 + dc * 128:cc * W4 + (dc + 1) * 128],
                                hqb[:, cc * SQ + f0:cc * SQ + f0 + fw],
                                start=(cc == 0), stop=(cc == 4))
                    for gi, (f0, fw) in enumerate(qcs):
                        nc.vector.tensor_copy(
                            qeb[:, dc * SQ + f0:dc * SQ + f0 + fw],
                            pp[:, gi * 512:gi * 512 + fw])
                for dc in range(5):
                    pp = eps.tile([128, 1536], F32, tag="ppq", name="ppk")
                    for cc in range(5):
                        for gi, (f0, fw) in enumerate(kcs):
                            nc.tensor.matmul(
                                pp[:, gi * 512:gi * 512 + fw],
                                went[:, cc * W4 + C + dc * 128:cc * W4 + C + (dc + 1) * 128],
                                hkb[:, cc * SK + f0:cc * SK + f0 + fw],
                                start=(cc == 0), stop=(cc == 4))
                    for gi, (f0, fw) in enumerate(kcs):
                        nc.vector.tensor_copy(
                            keb[:, dc * SK + f0:dc * SK + f0 + fw],
                            pp[:, gi * 512:gi * 512 + fw])
            with tc.tile_pool(name="entpv", bufs=2, space="PSUM") as eps:
                for kt in range(TEM):
                    pp = eps.tile([128, C], F32, tag="pp")
                    for o0, w in ((0, 512), (512, 128)):
                        for cc in range(5):
                            nc.tensor.matmul(
                                pp[:, o0:o0 + w],
                                hkb[:, cc * SK + kt * 128:cc * SK + (kt + 1) * 128],
                                went[:, cc * W4 + 2 * C + o0:cc * W4 + 2 * C + o0 + w],
                                start=(cc == 0), stop=(cc == 4))
                    nc.vector.tensor_copy(veb[:, kt * C:(kt + 1) * C], pp[:])

            oTe = ep.tile([128, 5 * SQ], BF16, tag="oTe")
            PTe = ep.tile([128, TEM * 512], BF16, tag="PTe")
            den_s = ep.tile([1, 512], F32, tag="den_s")
            rec_s = ep.tile([1, 512], F32, tag="rec_s")
            for q0, qw in _chunks(SQ):
                with tc.tile_pool(name="entsc", bufs=2, space="PSUM") as scp, \
                     tc.tile_pool(name="entav", bufs=1, space="PSUM") as avp:
                    pave = avp.tile([128, 5 * 512], F32, tag="pave")
                    pden = avp.tile([1, 512], F32, tag="pden")
                    for kt in range(TEM):
                        pse = scp.tile([128, 512], F32, tag="pse")
                        for dc in range(5):
                            nc.tensor.matmul(
                                pse[:, 0:qw],
                                keb[:, dc * SK + kt * 128:dc * SK + (kt + 1) * 128],
                                qeb[:, dc * SQ + q0:dc * SQ + q0 + qw],
                                start=(dc == 0), stop=(dc == 4))
                        nc.scalar.activation(PTe[:, kt * qw:(kt + 1) * qw],
                                             pse[:, 0:qw], EXP)
                    for kt in range(TEM):
                        for dc in range(5):
                            # dc*512: one PSUM bank per concurrent accum group
                            nc.tensor.matmul(
                                pave[:, dc * 512:dc * 512 + qw],
                                veb[:, kt * C + dc * 128:kt * C + (dc + 1) * 128],
                                PTe[:, kt * qw:(kt + 1) * qw],
                                start=(kt == 0), stop=(kt == TEM - 1))
                        nc.tensor.matmul(pden[:, 0:qw], ones_bf[:],
                                         PTe[:, kt * qw:(kt + 1) * qw],
                                         start=(kt == 0), stop=(kt == TEM - 1))
                    nc.vector.tensor_scalar(den_s[0:1, 0:qw], pden[:, 0:qw],
                                            entc_sb[0:1, 0:1], None, op0=ADD)
                    nc.vector.reciprocal_approx_fast(rec_s[0:1, 0:qw],
                                                     den_s[0:1, 0:qw])
                    rec_b = ep.tile([1, 512], BF16, tag="rec_b")
                    nc.vector.tensor_copy(rec_b[0:1, 0:qw], rec_s[0:1, 0:qw])
                    pB = scp.tile([128, 512], F32, tag="pse", name="pB")
                    nc.tensor.matmul(pB[:, 0:qw], ones_b1[0:1, :],
                                     rec_b[0:1, 0:qw], start=True, stop=True)
                    pBs = ep.tile([128, 512], F32, tag="pBs")
                    nc.vector.tensor_copy(pBs[:, 0:qw], pB[:, 0:qw])
                    for dc in range(5):
                        nc.vector.tensor_mul(
                            oTe[:, dc * SQ + q0:dc * SQ + q0 + qw],
                            pave[:, dc * 512:dc * 512 + qw], pBs[:, 0:qw])
            # ent Wo projection -> eout
            eoutb = ep.tile([128, NQT * C], BF16, tag="eoutb")
            with tc.tile_pool(name="entwo", bufs=2, space="PSUM") as ewp:
                for st in range(NQT):
                    pw = ewp.tile([128, C], F32, tag="pwe")
                    for o0, w in ((0, 512), (512, 128)):
                        for cc in range(5):
                            nc.tensor.matmul(
                                pw[:, o0:o0 + w],
                                oTe[:, cc * SQ + st * 128:cc * SQ + (st + 1) * 128],
                                went[:, cc * W4 + 3 * C + o0:cc * W4 + 3 * C + o0 + w],
                                start=(cc == 0), stop=(cc == 4))
                    nc.scalar.activation(eoutb[:, st * C:(st + 1) * C], pw[:],
                                         COPY)
                for st in range(NQT):
                    nc.sync.dma_start(eout_d.ap()[st * 128:(st + 1) * 128, :],
                                      eoutb[:, st * C:(st + 1) * C])

        # ============ orig + out branches (1 head each per core) ========
        main = ctx.enter_context(tc.tile_pool(name="main", bufs=1))
        qTo = main.tile([D, Stot], BF16, tag="qTo")
        kTo = main.tile([D, Stot], BF16, tag="kTo")
        qTu = main.tile([D, Stot], BF16, tag="qTu")
        kTu = main.tile([D, Stot], BF16, tag="kTu")
        vso = main.tile([128, Ttot * 97], BF16, tag="vso")
        vsu = main.tile([128, Ttot * 97], BF16, tag="vsu")
        hsTo = main.tile([D, Stot], BF16, tag="hsTo")
        hsTu = main.tile([D, Stot], BF16, tag="hsTu")
        hsTs = main.tile([D, Stot], BF16, tag="hsTs")
        PTs = [main.tile([128, TEM * 512], BF16, tag="PT0", name="PT0"),
               main.tile([128, TEM * 512], BF16, tag="PT1", name="PT1")]
        nc.gpsimd.memset(vso[:], 1.0)
        nc.gpsimd.memset(vsu[:], 1.0)

        with tc.tile_pool(name="pjps", bufs=2, space="PSUM") as pjp:
            fcs = _chunks(Stot)
            for dst, wcol, scl in ((qTo, 0, True), (kTo, 80, False),
                                   (qTu, 160, True), (kTu, 240, False)):
                for g0 in range(0, len(fcs), 3):
                    grp = fcs[g0:g0 + 3]
                    pq = pjp.tile([D, 1536], F32, tag="pq")
                    for cc in range(5):
                        for gi, (f0, fw) in enumerate(grp):
                            nc.tensor.matmul(
                                pq[:, gi * 512:gi * 512 + fw],
                                wh[:, cc * 480 + wcol:cc * 480 + wcol + D],
                                hTb[:, cc * Stot + f0:cc * Stot + f0 + fw],
                                start=(cc == 0), stop=(cc == 4))
                    for gi, (f0, fw) in enumerate(grp):
                        nc.vector.tensor_copy(dst[:, f0:f0 + fw],
                                              pq[:, gi * 512:gi * 512 + fw])
            # out-branch boundary tiles: keys n0[e]..T0[e]*128 are im==1 and
            # must not contribute -> zero their k columns and v rows
            bnd = {}
            for e in range(NE):
                if T[e] == 0 or n0[e] == 0 or n0[e] % 128 == 0:
                    continue
                bnd[off[e] // 128 + T0[e] - 1] = n0[e] % 128
                nc.vector.memset(kTu[:, off[e] + n0[e]:off[e] + T0[e] * 128],
                                 0.0)
            for kt in range(Ttot):
                pv = pjp.tile([128, 160], F32, tag="pv")
                for cc in range(5):
                    nc.tensor.matmul(
                        pv[:],
                        hTb[:, cc * Stot + kt * 128:cc * Stot + (kt + 1) * 128],
                        wh[:, cc * 480 + 320:cc * 480 + 480],
                        start=(cc == 0), stop=(cc == 4))
                nc.vector.tensor_copy(vso[:, kt * 97:kt * 97 + 80], pv[:, 0:80])
                if kt in bnd:
                    nc.vector.memset(vsu[:, kt * 97:kt * 97 + 80], 0.0)
                    nc.vector.tensor_copy(vsu[0:bnd[kt], kt * 97:kt * 97 + 80],
                                          pv[0:bnd[kt], 80:160])
                else:
                    nc.vector.tensor_copy(vsu[:, kt * 97:kt * 97 + 80],
                                          pv[:, 80:160])

        # attention + Wo partials + per-entity ReduceScatter
        atx = ExitStack()
        with atx:
            psp = atx.enter_context(tc.tile_pool(name="psp", bufs=2, space="PSUM"))
            avp = atx.enter_context(tc.tile_pool(name="avp", bufs=2, space="PSUM"))
            wop = atx.enter_context(tc.tile_pool(name="wop", bufs=2, space="PSUM"))
            sb2 = atx.enter_context(tc.tile_pool(name="sb2", bufs=2))
            # deep pool so Wo evictions never wait for P-write DMAs that
            # are queued behind a running collective
            pbp = atx.enter_context(tc.tile_pool(name="pbp", bufs=2))
            eorder = sorted([e for e in range(NE) if T[e] > 0],
                            key=lambda e: (-T[e], e))
            # flat job list: (e, branch params, chunk) with a marker on the
            # last chunk of each entity
            jobs = []
            for e in eorder:
                for br, qT, kT, vs, hsT, nkt, corr in (
                        ("o", qTo, kTo, vso, hsTo, T[e],
                         float(n[e] - T[e] * 128)),
                        ("u", qTu, kTu, vsu, hsTu, T0[e],
                         float(n0[e] - T0[e] * 128))):
                    for q0, qw in _chunks(T[e] * 128):
                        jobs.append([e, br, qT, kT, vs, hsT, nkt, corr,
                                     q0, qw, False])
                jobs[-1][10] = True  # entity boundary

            def emit_scores(job, PT):
                e, br, qT, kT, vs, hsT, nkt, corr, q0, qw, last = job
                oe = off[e]
                for kt in range(nkt):
                    ps = psp.tile([128, 512], F32, tag="ps")
                    nc.tensor.matmul(
                        ps[:, 0:qw],
                        kT[:, oe + kt * 128:oe + (kt + 1) * 128],
                        qT[:, oe + q0:oe + q0 + qw],
                        start=True, stop=True)
                    nc.scalar.activation(PT[:, kt * qw:(kt + 1) * qw],
                                         ps[:, 0:qw], EXP)

            def emit_av(job, PT):
                e, br, qT, kT, vs, hsT, nkt, corr, q0, qw, last = job
                oe = off[e]
                pav = avp.tile([128, 512], F32, tag="pav")
                for kt in range(nkt):
                    nc.tensor.matmul(
                        pav[0:97, 0:qw],
                        vs[:, (oe // 128 + kt) * 97:(oe // 128 + kt) * 97 + 97],
                        PT[:, kt * qw:(kt + 1) * qw],
                        start=(kt == 0), stop=(kt == nkt - 1))
                dn = sb2.tile([1, 512], F32, tag="dn")
                rc = sb2.tile([1, 512], F32, tag="rc")
                rcb = sb2.tile([1, 512], BF16, tag="rcb")
                nc.vector.tensor_scalar(dn[0:1, 0:qw], pav[96:97, 0:qw],
                                        corr, None, op0=ADD)
                nc.vector.reciprocal_approx_fast(rc[0:1, 0:qw], dn[0:1, 0:qw])
                nc.vector.tensor_copy(rcb[0:1, 0:qw], rc[0:1, 0:qw])
                pB = psp.tile([128, 512], F32, tag="ps", name="pBm")
                nc.tensor.matmul(pB[0:D, 0:qw], ones_b1[0:1, 0:D],
                                 rcb[0:1, 0:qw], start=True, stop=True)
                pBs = sb2.tile([D, 512], F32, tag="pBs")
                nc.vector.tensor_copy(pBs[:, 0:qw], pB[0:D, 0:qw])
                nc.vector.tensor_mul(hsT[:, oe + q0:oe + q0 + qw],
                                     pav[0:D, 0:qw], pBs[:, 0:qw])
                if br == "u":
                    nc.vector.tensor_add(hsTs[:, oe + q0:oe + q0 + qw],
                                         hsTo[:, oe + q0:oe + q0 + qw],
                                         hsTu[:, oe + q0:oe + q0 + qw])

            def emit_wo_rs(e):
                oe = off[e]
                pbatch = pbp.tile([128, TEM * C], BF16, tag="pbatch")
                for st in range(T[e]):
                    gt = oe // 128 + st
                    pw = wop.tile([128, C], F32, tag="pw")
                    for o0, w in ((0, 512), (512, 128)):
                        nc.tensor.matmul(pw[:, o0:o0 + w],
                                         hsTs[:, gt * 128:(gt + 1) * 128],
                                         woh_sb[:, o0:o0 + w],
                                         start=True, stop=True)
                    nc.scalar.activation(pbatch[:, st * C:(st + 1) * C],
                                         pw[:], COPY)
                h1 = (T[e] // 2) * C
                nc.sync.dma_start(P_d[e].ap()[:, 0:h1], pbatch[:, 0:h1])
                nc.sync.dma_start(P_d[e].ap()[:, h1:T[e] * C],
                                  pbatch[:, h1:T[e] * C])
                nc.gpsimd.collective_compute(
                    "ReduceScatter", ADD,
                    replica_groups=[list(range(NCORES))],
                    ins=[P_d[e].ap()[:]],
                    outs=[red_d[e].ap()[:]])
                nc.sync.dma_start(out_d[e].ap()[:], red_d[e].ap()[:])

            # software pipeline: scores(i+1) before av(i) so the exp latency
            # hides behind PE work; Wo/RS fire at entity boundaries
            for i, job in enumerate(jobs):
                emit_scores(job, PTs[i % 2])
                if i > 0:
                    emit_av(jobs[i - 1], PTs[(i - 1) % 2])
                    if jobs[i - 1][10]:
                        emit_wo_rs(jobs[i - 1][0])
            emit_av(jobs[-1], PTs[(len(jobs) - 1) % 2])
            emit_wo_rs(jobs[-1][0])


def _plan(mask, inpainting_mask):
    m = np.asarray(mask[0, 0], np.int64)[::8, ::8].reshape(-1)
    im = np.asarray(inpainting_mask[0, 0], np.int64)[::8, ::8].reshape(-1)
    NE = int(m.max()) + 1
    n = [int((m == e).sum()) for e in range(NE)]
    n0 = [int(((m == e) & (im == 0)).sum()) for e in range(NE)]
    for e in range(NE):
        assert n[e] == 0 or n0[e] > 0, "empty outside-key block unsupported"
    T = [(x + 127) // 128 for x in n]
    order = np.lexsort((im, m))
    off = np.cumsum([0] + [t * 128 for t in T])
    pos = np.concatenate([off[e] + np.arange(n[e]) for e in range(NE)
                          if n[e] > 0]).astype(np.int64)
    assign = tuple(_assign_cores(T))
    cfg = (tuple(T), tuple(n), tuple(n0), assign)
    return cfg, order, pos, off


def build_in_maps(hidden_states, mask, inpainting_mask, Wq, Wk, Wv,
                  Wq_ent, Wk_ent, Wv_ent, Wq_out, Wk_out, Wv_out, Wo):
    cfg, order, pos, off = _plan(mask, inpainting_mask)
    T, n, n0, assign = cfg
    TEM = max(T)
    NQT = max(a[2] for a in assign)
    Stot = sum(T) * 128
    SK, SQ = TEM * 128, NQT * 128

    h = np.asarray(hidden_states[0], np.float32)
    hp = np.zeros((Stot, C), np.float32)
    hp[pos] = h[order]
    hTb = np.ascontiguousarray(hp.T).astype(BF)

    def t(W):
        return np.asarray(W, np.float32).T

    went = np.ascontiguousarray(np.concatenate(
        [t(Wq_ent) * SCALE_E, t(Wk_ent), t(Wv_ent), t(Wo)], axis=1)).astype(BF)
    WoT = t(Wo)

    in_maps = []
    for i in range(NCORES):
        hd = slice(D * i, D * (i + 1))
        whead = np.ascontiguousarray(np.concatenate(
            [t(Wq)[:, hd] * SCALE_H, t(Wk)[:, hd],
             t(Wq_out)[:, hd] * SCALE_H, t(Wk_out)[:, hd],
             t(Wv)[:, hd], t(Wv_out)[:, hd]], axis=1)).astype(BF)
        e, t0, nt = assign[i]
        hq = np.zeros((C, SQ), BF)
        hq[:, :nt * 128] = hTb[:, off[e] + t0 * 128:off[e] + (t0 + nt) * 128]
        hk = np.zeros((C, SK), BF)
        hk[:, :T[e] * 128] = hTb[:, off[e]:off[e] + T[e] * 128]
        in_maps.append({
            "hT": hTb, "whead": whead, "went": went,
            "woh": np.ascontiguousarray(WoT[hd, :]).astype(BF),
            "hq": hq, "hk": hk,
            "entc": np.array([[n[e] - TEM * 128]], np.float32),
        })
    _cache["plan"] = (cfg, order, pos, off, hp)
    return in_maps


def kernel(**inputs):
    in_maps = build_in_maps(**inputs)
    cfg, order, pos, off, hp = _cache["plan"]
    T, n, n0, assign = cfg
    Stot = sum(T) * 128
    key = ("nc", cfg)
    if key not in _cache:
        _cache["nc"] = _build(cfg)
        _cache[key] = _cache["nc"]
    res = run_bass_kernel_spmd(_cache[key], in_maps, list(range(NCORES)),
                               trace=False)
    acc = np.zeros((Stot, C), np.float32)
    for e in range(len(T)):
        if T[e] == 0:
            continue
        acc_e = acc[off[e]:off[e] + T[e] * 128].reshape(T[e], 128, C)
        for i in range(NCORES):
            blk = np.asarray(res.results[i][f"o{e}"],
                             np.float32).reshape(16, T[e], C)
            acc_e[:, 16 * i:16 * (i + 1), :] = blk.transpose(1, 0, 2)
    for i, (e, t0, nt) in enumerate(assign):
        q0 = off[e] + t0 * 128
        acc[q0:q0 + nt * 128] += np.asarray(
            res.results[i]["eout"][:nt * 128], np.float32)
    acc += hp
    out = np.empty((S, C), np.float32)
    out[order] = acc[pos]
    return out.reshape(1, S, C)

